# revision 20
# baseline (speedup 1.0000x reference)
"""Trainium2 Bass kernel for nn_AttentionMLP (embedding + 4-head attention + MLP head).

Sharding: data-parallel over batch B=8 across 8 NeuronCores (core b <- batch b).
Weights replicated; no collectives.  Per-core pipeline (S=2048, E=128, H=4, W=8),
designed so the ACT engine (exp over 4*2048*2048 scores ~ 118us) is the only
non-overlapped cost:

  1. h = emb_table[x[b]] + pos_enc -> hT [E=128, S=2048], built per 4-tile group:
     batched indirect-DMA gather, 4 PE transposes into one PSUM bank, one DVE add.
  2. qT/kT [128, 2048] f32r: head h at partitions 32h..32h+8 (padded projection
     weights); one N=512 matmul + one DVE bias-add per group.
  3. v' per s-tile [128, 32/head] bf16: col 32h is the softmax-denominator ones
     column (via bias), cols 32h+1..8 hold Wv, rest zero-padded so the ctx
     matmul writes every PSUM row (keeps normalization branch-free).
  4. Attention per (chunk c of 512 q, k-tile t): scores split across two 2-bank
     PSUM tiles (heads 01 -> scpsA, heads 23 -> scpsB) so the PE refills one
     while ACT exps the other; exp outputs bf16; 4 col-tiled ctx matmuls
     accumulate attn@v' (and the denominators) into one PSUM bank.
  5. Normalize: DVE copy ctx->SBUF, PE selection-matmul broadcasts the denom
     row across each head band, reciprocal_approx_fast, one DVE multiply.
     Wo via 4 matmuls into one bank, one DVE bias-add -> o chunk (bf16).
  6. Final [S*E] @ W1 GEMV: W1 pre-laid [8, 128, 3072] bf16 on host, fully
     prefetched to SBUF; per s-tile one DVE broadcast-multiply + one reduce;
     cross-partition finish via PE matmul against ones; add b1.
"""

import numpy as np

VOCAB, E, S, B, H, W = 50257, 128, 2048, 8, 4, 8
P = 128
NT = S // P            # 16 s-tiles
NC_ = 8                # cores
SQC = 512              # q chunk size == group size
NCH = S // SQC         # 4 chunks / groups
HB = 32                # padded v' features per head (1 ones + 8 v + 23 zero)
SCALE = float(1.0 / np.float32(np.sqrt(8.0)))
REP = 1  # timing aid: repeat the whole per-core body REP times in one NEFF

_CACHE = {}


def _pos_encoding_T():
    pos = np.arange(S, dtype=np.float32)[:, None]
    div = np.exp(np.arange(0, E, 2, dtype=np.float32) * (-np.log(10000.0) / E))
    pe = np.zeros((S, E), dtype=np.float32)
    pe[:, 0::2] = np.sin(pos * div)
    pe[:, 1::2] = np.cos(pos * div)
    return np.ascontiguousarray(pe.T)  # [E=128, S]


def _emit(nc, tc, d, mybir, bass, make_identity):
    from contextlib import ExitStack

    f32 = mybir.dt.float32
    f32r = mybir.dt.float32r
    bf16 = mybir.dt.bfloat16
    AOT = mybir.AluOpType
    EXP = mybir.ActivationFunctionType.Exp

    with ExitStack() as ctx:
        sb = ctx.enter_context(tc.tile_pool(name="sb", bufs=1))
        pp = ctx.enter_context(tc.tile_pool(name="pp", bufs=1, space="PSUM"))

        # ---- constants / inputs to SBUF ----
        x_sb = sb.tile([P, NT], mybir.dt.int32)
        nc.sync.dma_start(x_sb, d["x_idx"].ap())
        zeros128 = sb.tile([P, P], f32)
        nc.gpsimd.memset(zeros128, 0.0)
        # preload the exp table right away (input needs no DMA)
        warm = sb.tile([1, 8], f32, tag="warm", bufs=1)
        nc.scalar.activation(warm, zeros128[0:1, 0:8], EXP)
        identity = sb.tile([P, P], f32)
        make_identity(nc, identity)
        ones_col = sb.tile([P, 1], f32)
        nc.gpsimd.memset(ones_col, 1.0)
        zero_idx = sb.tile([P, 1], mybir.dt.int32)
        nc.gpsimd.memset(zero_idx, 0)

        peT_sb = sb.tile([P, S], f32)
        for g in range(NCH):
            sl = slice(g * SQC, (g + 1) * SQC)
            nc.sync.dma_start(peT_sb[:, sl], d["peT"].ap()[:, sl])
        wq_sb = sb.tile([P, P], f32)
        nc.sync.dma_start(wq_sb, d["wq_pad"].ap())
        wk_sb = sb.tile([P, P], f32)
        nc.sync.dma_start(wk_sb, d["wk_pad"].ap())
        bqp_sb = sb.tile([P, 1], f32)
        nc.sync.dma_start(bqp_sb, d["bq_pad"].ap())
        bkp_sb = sb.tile([P, 1], f32)
        nc.sync.dma_start(bkp_sb, d["bk_pad"].ap())
        wv_sb = sb.tile([P, P], f32)
        nc.sync.dma_start(wv_sb, d["wv_pack"].ap())
        bvb_sb = sb.tile([P, 4 * P], f32)
        nc.sync.dma_start(bvb_sb, d["bv_bcast"].ap())
        wo_sb = sb.tile([P, E], f32)
        nc.sync.dma_start(wo_sb, d["wo_pad"].ap())
        bob_sb = sb.tile([P, 4 * E], f32)
        nc.sync.dma_start(bob_sb, d["bo_bcast"].ap())
        sel_sb = sb.tile([P, P], f32)
        nc.sync.dma_start(sel_sb, d["sel"].ap())
        b1_sb = sb.tile([12, 1], f32)
        nc.sync.dma_start(b1_sb, d["b1c"].ap())
        rep_sb = sb.tile([1, 8 * REP], f32)
        nc.sync.dma_start(rep_sb, d["rep_tag"].ap())

        # W1 fully resident: [128, 16*12*128] bf16 = 48KB/partition, streamed
        # in 8 chunks so it interleaves with the gathers on the DMA engines.
        w1_all = sb.tile([P, NT * 12 * E], bf16)
        w1_ap = d["w1b"].ap()
        CW = 2 * 12 * E  # columns per chunk (2 s-tiles)
        for j in range(8):
            nc.sync.dma_start(w1_all[:, j * CW : (j + 1) * CW], w1_ap[j])

        emb_ap = d["emb_table"].ap()

        # q chunks (start, size): small first chunk starts the exp stream
        # early (needs only 2 hT tiles); small last chunk shrinks the
        # serial norm+GEMV tail after the final exp.
        CHUNKS = [(0, 512), (512, 512), (1024, 512), (1536, 512)]

        D_acc = sb.tile([P, 12], f32)  # GEMV accumulator, reset per rep

        def emit_group(nc_, hT, qT, kT, v_sb, emb_ts, t0, ntile):
            """hT/kT/qT/v' for tiles t0..t0+ntile-1 (ntile*128 columns)."""
            gs = slice(t0 * P, (t0 + ntile) * P)
            w = ntile * P
            trp = pp.tile([P, SQC], f32, tag="psX", bufs=2)
            for i in range(ntile):
                nc.tensor.transpose(
                    trp[:, i * P : (i + 1) * P], emb_ts[t0 + i], identity
                )
            nc.vector.tensor_tensor(
                out=hT[:, gs], in0=trp[:, 0:w], in1=peT_sb[:, gs], op=AOT.add
            )
            kps = pp.tile([P, SQC], f32, tag="psX", bufs=2)
            nc.tensor.matmul(kps[:, 0:w], wk_sb, hT[:, gs], start=True, stop=True)
            nc.vector.tensor_scalar_add(kT[:, gs], kps[:, 0:w], bkp_sb[:, 0:1])
            qps = pp.tile([P, SQC], f32, tag="psX", bufs=2)
            nc.tensor.matmul(qps[:, 0:w], wq_sb, hT[:, gs], start=True, stop=True)
            nc.vector.tensor_scalar_add(qT[:, gs], qps[:, 0:w], bqp_sb[:, 0:1])
            vps = pp.tile([P, SQC], f32, tag="psX", bufs=2)
            for i in range(ntile):
                t = t0 + i
                nc.tensor.matmul(
                    vps[:, i * P : (i + 1) * P],
                    hT[:, t * P : (t + 1) * P], wv_sb,
                    start=True, stop=True,
                )
            nc.vector.tensor_tensor(
                out=v_sb[:, gs], in0=vps[:, 0:w], in1=bvb_sb[:, 0:w], op=AOT.add
            )

        def attention(qT, kT, v_sb, qs, qn):
            ctxc = pp.tile([P, SQC], f32, tag="ctx", bufs=2)
            # open one accumulation group for the bank (writes 0s); the
            # per-head col-tiled ctx matmuls then accumulate into it.
            nc.tensor.matmul(
                ctxc[:, 0:qn], zeros128, peT_sb[:, 0:qn], start=True, stop=False
            )
            cs = slice(qs, qs + qn)
            for t in range(NT):
                ts = slice(t * P, (t + 1) * P)
                exps = sb.tile([P, H * SQC], bf16, tag="exps", bufs=3)
                for half, sc_tag in ((0, "scpsA"), (1, "scpsB")):
                    # fixed 512-column head slots so each score matmul
                    # stays inside one PSUM bank even when qn < 512
                    scps = pp.tile([P, 2 * SQC], f32, tag=sc_tag, bufs=1)
                    for hh in range(2):
                        h = 2 * half + hh
                        nc.tensor.matmul(
                            scps[:, hh * SQC : hh * SQC + qn],
                            kT[32 * h : 32 * h + W, ts],
                            qT[32 * h : 32 * h + W, cs],
                            start=True, stop=True,
                            tile_position=(32 * h, 0),
                        )
                    nc.scalar.activation(
                        exps[:, half * 2 * qn : (half + 1) * 2 * qn]
                        .rearrange("p (s q) -> p s q", s=2),
                        scps.rearrange("p (s q) -> p s q", s=2)[:, :, 0:qn],
                        EXP,
                    )
                for h in range(H):
                    nc.tensor.matmul(
                        ctxc[32 * h : 32 * h + HB, 0:qn],
                        v_sb[:, t * P + 32 * h : t * P + 32 * h + HB],
                        exps[:, h * qn : (h + 1) * qn],
                        start=False, stop=False,
                        tile_position=(0, 32 * h),
                    )
            # close the bank-wide accumulation group (adds zeros)
            nc.tensor.matmul(
                ctxc[:, 0:qn], zeros128, peT_sb[:, 0:qn], start=False, stop=True
            )
            return ctxc

        def finish(item):
            ctxc, qs, qn, rep_end = item
            # ---- 5. normalize + Wo ----
            ctxS = sb.tile([P, SQC], f32, tag="ctxS", bufs=2)
            nc.vector.tensor_copy(ctxS[:, 0:qn], ctxc[:, 0:qn])
            dps = pp.tile([P, SQC], f32, tag="psX", bufs=2)
            nc.tensor.matmul(
                dps[:, 0:qn], sel_sb, ctxS[:, 0:qn], start=True, stop=True
            )
            rcp = sb.tile([P, SQC], f32, tag="rcp", bufs=2)
            nc.vector.reciprocal_approx_fast(out=rcp[:, 0:qn], in_=dps[:, 0:qn])
            ctxN = sb.tile([P, SQC], f32, tag="ctxN", bufs=2)
            nc.vector.tensor_tensor(
                out=ctxN[:, 0:qn], in0=ctxS[:, 0:qn], in1=rcp[:, 0:qn],
                op=AOT.mult,
            )
            ops = pp.tile([P, SQC], f32, tag="psX", bufs=2)
            for st in range(qn // P):
                nc.tensor.matmul(
                    ops[:, st * P : (st + 1) * P],
                    ctxN[:, st * P : (st + 1) * P],
                    wo_sb,
                    start=True, stop=True,
                )
            o_c = sb.tile([P, SQC], bf16, tag="o_c", bufs=2)
            nc.vector.tensor_tensor(
                out=o_c[:, 0:qn], in0=ops[:, 0:qn], in1=bob_sb[:, 0:qn],
                op=AOT.add,
            )
            # ---- 6. GEMV partials for this chunk's s-tiles ----
            D = D_acc
            for st in range(qn // P):
                s_abs = qs // P + st
                w1_sl = w1_all[:, s_abs * 12 * E : (s_abs + 1) * 12 * E]
                prod = sb.tile([P, 12 * E], bf16, tag="prod", bufs=2)
                nc.vector.tensor_tensor(
                    out=prod,
                    in0=w1_sl,
                    in1=o_c[:, st * P : (st + 1) * P]
                    .rearrange("p (o e) -> p o e", o=1)
                    .to_broadcast((P, 12, E)),
                    op=AOT.mult,
                )
                Dt = sb.tile([P, 12], f32, tag="Dt", bufs=2)
                nc.vector.tensor_reduce(
                    out=Dt, in_=prod.rearrange("p (j e) -> p j e", j=12),
                    axis=mybir.AxisListType.X, op=AOT.add,
                )
                if s_abs == 0:
                    nc.vector.tensor_copy(D, Dt)
                else:
                    nc.vector.tensor_tensor(out=D, in0=D, in1=Dt, op=AOT.add)
            if rep_end:
                # ---- final cross-partition reduce + b1 ----
                finps = pp.tile([P, SQC], f32, tag="psX", bufs=2)
                nc.tensor.matmul(
                    finps[0:12, 0:1], D, ones_col, start=True, stop=True
                )
                final_sb = sb.tile([12, 1], f32, tag="final_sb", bufs=1)
                nc.vector.tensor_tensor(
                    out=final_sb, in0=finps[0:12, 0:1], in1=b1_sb, op=AOT.add
                )
                nc.sync.dma_start(d["outv"].ap(), final_sb)

        # finish(chunk) is deferred until after attention(next chunk) — across
        # rep boundaries too — so the next chunk's score matmuls sit ahead of
        # the norm-block matmuls in the PE FIFO and the tail GEMV overlaps the
        # next rep's exp stream.
        pending = None
        for _rep in range(REP):
            hT = sb.tile([P, S], f32, tag="hT", bufs=1)
            qT = sb.tile([P, S], f32r, tag="qT", bufs=2)
            kT = sb.tile([P, S], f32r, tag="kT", bufs=2)
            v_sb = sb.tile([P, NT * P], bf16, tag="v_sb", bufs=2)

            # The first indirect DMA on the queue returns garbage rows on HW;
            # issue a sacrificial gather (offsets=0) before the real ones.
            if _rep == 0:
                sac = sb.tile([P, E], f32, tag="sac", bufs=1)
                nc.gpsimd.indirect_dma_start(
                    out=sac, out_offset=None, in_=emb_ap,
                    in_offset=bass.IndirectOffsetOnAxis(
                        ap=zero_idx[:, 0:1], axis=0
                    ),
                )
            emb_ts = []
            for t in range(NT):
                emb_t = sb.tile([P, P], f32, tag="emb", bufs=8)
                nc.gpsimd.indirect_dma_start(
                    out=emb_t, out_offset=None, in_=emb_ap,
                    in_offset=bass.IndirectOffsetOnAxis(
                        ap=x_sb[:, t : t + 1], axis=0
                    ),
                )
                emb_ts.append(emb_t)

            for t0, ntile in ((0, 4), (4, 4), (8, 4), (12, 4)):
                emit_group(nc, hT, qT, kT, v_sb, emb_ts, t0, ntile)

            for ci, (qs, qn) in enumerate(CHUNKS):
                ctxc = attention(qT, kT, v_sb, qs, qn)
                if pending is not None:
                    finish(pending)
                pending = (ctxc, qs, qn, ci == len(CHUNKS) - 1)
        finish(pending)


def _build():
    if "nc" in _CACHE:
        return _CACHE["nc"], _CACHE["drams"]
    import concourse.bass as bass
    import concourse.tile as tile
    import concourse.mybir as mybir
    from concourse import bacc
    from concourse.masks import make_identity

    f32 = mybir.dt.float32
    bf16 = mybir.dt.bfloat16
    nc = bacc.Bacc(
        "TRN2", target_bir_lowering=False, debug=False,
        enable_asserts=False, num_devices=NC_,
    )
    d = {}
    d["x_idx"] = nc.dram_tensor("x_idx", [P, NT], mybir.dt.int32, kind="ExternalInput")
    d["emb_table"] = nc.dram_tensor("emb_table", [VOCAB, E], f32, kind="ExternalInput")
    d["peT"] = nc.dram_tensor("peT", [P, S], f32, kind="ExternalInput")
    d["wq_pad"] = nc.dram_tensor("wq_pad", [P, P], f32, kind="ExternalInput")
    d["wk_pad"] = nc.dram_tensor("wk_pad", [P, P], f32, kind="ExternalInput")
    d["bq_pad"] = nc.dram_tensor("bq_pad", [P, 1], f32, kind="ExternalInput")
    d["bk_pad"] = nc.dram_tensor("bk_pad", [P, 1], f32, kind="ExternalInput")
    d["wv_pack"] = nc.dram_tensor("wv_pack", [P, P], f32, kind="ExternalInput")
    d["bv_bcast"] = nc.dram_tensor("bv_bcast", [P, 4 * P], f32, kind="ExternalInput")
    d["wo_pad"] = nc.dram_tensor("wo_pad", [P, E], f32, kind="ExternalInput")
    d["bo_bcast"] = nc.dram_tensor("bo_bcast", [P, 4 * E], f32, kind="ExternalInput")
    d["sel"] = nc.dram_tensor("sel", [P, P], f32, kind="ExternalInput")
    d["w1b"] = nc.dram_tensor("w1b", [8, P, 2 * 12 * E], bf16, kind="ExternalInput")
    d["b1c"] = nc.dram_tensor("b1c", [12, 1], f32, kind="ExternalInput")
    d["rep_tag"] = nc.dram_tensor("rep_tag", [1, 8 * REP], f32, kind="ExternalInput")
    d["outv"] = nc.dram_tensor("outv", [12, 1], f32, kind="ExternalOutput")

    with tile.TileContext(nc) as tc:
        _emit(nc, tc, d, mybir, bass, make_identity)
    nc.compile()
    _CACHE["nc"] = nc
    _CACHE["drams"] = d
    return nc, d


def host_prep(inputs):
    """Build the 8 per-core input maps from full inputs."""
    import ml_dtypes

    x = np.asarray(inputs["x"])
    emb_table = np.ascontiguousarray(np.asarray(inputs["emb_table"], dtype=np.float32))
    Wq = np.asarray(inputs["Wq"], dtype=np.float32)
    bq = np.asarray(inputs["bq"], dtype=np.float32)
    Wk = np.asarray(inputs["Wk"], dtype=np.float32)
    bk = np.asarray(inputs["bk"], dtype=np.float32)
    Wv = np.asarray(inputs["Wv"], dtype=np.float32)
    bv = np.asarray(inputs["bv"], dtype=np.float32)
    Wo = np.asarray(inputs["Wo"], dtype=np.float32)
    bo = np.asarray(inputs["bo"], dtype=np.float32)
    W1 = np.asarray(inputs["W1"], dtype=np.float32)
    b1 = np.asarray(inputs["b1"], dtype=np.float32)

    peT = _pos_encoding_T()

    wq_pad = np.zeros((P, P), np.float32)
    wk_pad = np.zeros((P, P), np.float32)
    bq_pad = np.zeros((P, 1), np.float32)
    bk_pad = np.zeros((P, 1), np.float32)
    for h in range(H):
        # 1/sqrt(W) softmax scale folded into the q projection
        wq_pad[:, 32 * h : 32 * h + W] = Wq[h] * SCALE
        wk_pad[:, 32 * h : 32 * h + W] = Wk[h]
        bq_pad[32 * h : 32 * h + W, 0] = bq[h] * SCALE
        bk_pad[32 * h : 32 * h + W, 0] = bk[h]

    # v' weights: col 32h = 0 (ones via bias), 32h+1..8 = Wv[h], rest zero
    wv_pack = np.zeros((P, P), np.float32)
    bv128 = np.zeros((P,), np.float32)
    for h in range(H):
        wv_pack[:, 32 * h + 1 : 32 * h + 1 + W] = Wv[h]
        bv128[32 * h + 1 : 32 * h + 1 + W] = bv[h]
        bv128[32 * h] = 1.0  # ones column -> softmax denominator at row 32h
    bv_bcast = np.tile(bv128, (P, 4)).astype(np.float32)

    wo_pad = np.zeros((P, E), np.float32)
    for h in range(H):
        # rows 32h and 32h+9..31 stay zero (denominator row + padding)
        wo_pad[32 * h + 1 : 32 * h + 1 + W, :] = Wo[h * W : (h + 1) * W, :]
    bo_bcast = np.tile(bo, (P, 4)).astype(np.float32)

    # selection matrix: broadcasts row 32h (denominator) over its 32-row band
    sel = np.zeros((P, P), np.float32)
    for h in range(H):
        sel[32 * h, 32 * h : 32 * h + 32] = 1.0

    # W1[(t*128+p)*128+e, j] -> [t, p, j, e] -> chunks [8, P, 2*12*E] bf16
    w1h = W1.reshape(NT, P, E, 12).transpose(0, 1, 3, 2)  # [t, p, j, e]
    w1b = np.ascontiguousarray(
        w1h.reshape(8, 2, P, 12 * E).transpose(0, 2, 1, 3).reshape(8, P, 2 * 12 * E)
    ).astype(ml_dtypes.bfloat16)
    b1c = b1.reshape(12, 1).astype(np.float32)

    shared = {
        "emb_table": emb_table, "peT": peT,
        "wq_pad": wq_pad, "wk_pad": wk_pad, "bq_pad": bq_pad, "bk_pad": bk_pad,
        "wv_pack": wv_pack, "bv_bcast": bv_bcast,
        "wo_pad": wo_pad, "bo_bcast": bo_bcast, "sel": sel,
        "w1b": w1b, "b1c": b1c,
        "rep_tag": np.zeros((1, 8 * REP), np.float32),
    }
    in_maps = []
    for b in range(B):
        x_idx = np.ascontiguousarray(
            x[b].reshape(NT, P).T.astype(np.int32)
        )  # [128, 16]: col t = indices for s-tile t
        in_maps.append({**shared, "x_idx": x_idx})
    return in_maps


def kernel(**inputs):
    from concourse import bass_utils
    from concourse.bass_interp import get_hw_module

    in_maps = host_prep(inputs)
    nc, _ = _build()
    old_m = nc.m
    nc.m = get_hw_module(nc.m)
    try:
        res = bass_utils.run_bass_kernel_spmd(
            nc, in_maps, core_ids=list(range(NC_))
        )
    finally:
        nc.m = old_m
    out = np.stack([r["outv"].reshape(12) for r in res.results], axis=0)
    return out.astype(np.float32)
